# revision 40
# baseline (speedup 1.0000x reference)
"""NVFP4 block-quantized linear layer (x @ w.T + bias) on 8 Trainium2 cores.

Reference semantics (reference.py): both activations and weights are
block-quantized along K (blocks of 16) to fp4-e2m1 with e4m3 scales
(scale = absmax/6, round-to-nearest), dequantized, then matmul with fp32
accumulation, cast to bf16, plus bf16 bias.

Device strategy (per core, 2-way M x 4-way N grid), v2:
  - single-row-tile quant chains on VectorE (finer supply granularity),
    with the small per-block scale ops batched per two chains; exponent
    reciprocal via exact XOR/SUB bit trick (replaces a 3us RECIPROCAL).
  - build-time wavefront schedule interleaves quant chains and matmul
    cells so the PE does not starve on quant supply (the old sb-major
    order stalled ~450us waiting on W quant).
  - bias pre-filled into PSUM by ScalarE; matmuls accumulate onto it
    (start=False), dropping the DVE bias pass.
  - transposes grouped 4-per-PSUM-tile with one ScalarE evac each (4x
    fewer evacs; removes the PE-behind-ACT micro-stalls).
  - wdqT DRAM roundtrip in [NTn][KC][128,nt] layout: wt loads batched
    6-kc per DMA issue, W-transpose writebacks one strided DMA per row
    tile (3x fewer SP DMA issues).
  - xdqT resident in a 16-slot SBUF window (4 super-blocks); stage B
    super-blocks reuse slots via Tile WAR tracking.

Measured on 8 trn2 cores: ~1.42-1.47 ms HW exec (baseline 2.18 ms),
rel err ~3e-3 (tolerance 2e-2; the delta vs the old 1.6e-4 is the
single-rounded bias add from the PSUM prefill).
"""

import numpy as np
import ml_dtypes

f32 = np.float32
bf16 = ml_dtypes.bfloat16

# ---------------------------------------------------------------------------
# problem geometry (hardcoded; harness calls kernel() with these full shapes)
B, T, K = 2, 4096, 3072
N = 12288
M = B * T                      # 8192
GRID_M, GRID_N = 2, 4          # 8 cores
M_CORE = M // GRID_M           # 4096
N_CORE = N // GRID_N           # 3072
NUM_CORES = GRID_M * GRID_N

CH1 = float(1.5 * 2**22)
RCP6 = float(f32(1.0) / f32(6.0))

_BUILT = None


# ---------------------------------------------------------------------------
def _register_custom_ops():
    """Register the two fp4-rounding custom DVE ops (idempotent)."""
    import concourse.dve_ops as dve_ops
    from concourse.dve_ops import DveOp, OPS, _SUB_OPCODE_FOR_NAME, _CUSTOM_DVE_ROW_BASE
    from concourse.dve_spec import (
        Spec, Src0, Src1, C0, C1, Zero, One, AluOp, Bin,
        maxx, minn, select, lower, _has_src1,
    )
    from concourse.dve_uop import DveOpSpec

    def _norm2(in0, in1):
        in0 = np.asarray(in0)
        in1 = np.asarray(in1)
        if in1.size != in0.size:
            in1 = np.broadcast_to(in1, in0.shape)
        return in0, np.ascontiguousarray(in1).reshape(in0.shape)

    def _ref_fp4_pre(in0, in1, s0, s1, imm2=None):
        in0, in1 = _norm2(in0, in1)
        m = (in0.astype(f32) * in1.astype(f32)).astype(f32)
        s2 = (m * m).astype(f32)
        ch = np.where(
            s2 < f32(4.0), f32(CH1),
            ((f32(1.0) + (s2 >= f32(16.0)).astype(f32)) * f32(1.5 * 2**23)).astype(f32),
        ).astype(f32)
        return (m + ch).astype(f32)

    def _ref_fp4_fin(in0, in1, s0, s1, imm2=None):
        in0, in1 = _norm2(in0, in1)
        qpre = np.ascontiguousarray(in0.astype(f32))
        pe = (qpre.view(np.uint32) & np.uint32(0x7F800000)).view(f32)
        d1 = (qpre - pe).astype(f32)
        q2 = ((d1 + d1).astype(f32) - pe).astype(f32)
        qc = np.maximum(np.minimum(q2, f32(12.0)), f32(-12.0))
        return (qc * in1.astype(f32)).astype(f32)

    def build_pre():
        SIXTEEN = C0 * C0
        Ch2x = C1 + C1
        m = Src0 * Src1
        s2 = m * m
        c2 = s2 >= SIXTEEN
        inner = (c2 + One) * Ch2x
        c1 = s2 < C0
        outer = select(c1, C1, inner)
        return Spec(body=m + outer, reference=_ref_fp4_pre)

    def build_fin():
        pe = Bin(AluOp.BITWISE_AND, Src0, C0)
        d1 = Src0 - pe
        q2 = (d1 + d1) - pe
        qc = maxx(minn(q2, C1), Zero - C1)
        return Spec(body=qc * Src1, reference=_ref_fp4_fin)

    def register(name, spec):
        if name in _SUB_OPCODE_FOR_NAME:
            for op in OPS:
                if op.name == name:
                    return op
            raise RuntimeError(name)
        row = _CUSTOM_DVE_ROW_BASE + len(OPS)
        assert row < 0x20
        shas = {}
        for ver in ("v3", "v4"):
            try:
                uops = lower(spec, ver=ver)
            except Exception:
                continue
            shas[ver] = DveOpSpec(
                name=name, opcode=row, uops=uops, rd1_en=_has_src1(spec)
            ).sha(ver)
        op = DveOp(name, spec, subdim=False, uops_sha=shas)
        OPS.append(op)
        _SUB_OPCODE_FOR_NAME[name] = row
        dve_ops.CUSTOM_DVE_SPECS[name] = spec
        return op

    return register("FP4_PRE_ANT", build_pre()), register("FP4_FIN_ANT", build_fin())


# ---------------------------------------------------------------------------
def _patch_tile_drain():
    """The TileContext tail drain attaches one sem-wait per live logical
    processor to a single SP Drain instruction; this walrus build caps sync
    waits per instruction at 2 ("Too many sync wait commands").  Split the
    overflow waits onto preceding single-wait SP nops (sound: all waits still
    complete before the post-drain all-engine barrier / sem reset)."""
    from concourse import tile as tile_mod
    import concourse.mybir as mybir
    from concourse.vector_clock import ScopedClock

    if getattr(tile_mod.TileContext, "_ant_drain_patched", False):
        return

    def _drain_and_barrier(self, tick_clock, wait_clock):
        nc = self.nc
        probe = nc.sync.nop()
        wait_clock.add_sem_waits(
            probe.ins, ScopedClock({None: tick_clock.global_clock})
        )
        si = probe.ins.sync_info
        waits = list(si.on_wait) if si is not None and si.on_wait else []
        if len(waits) > 1:
            probe.ins.sync_info = mybir.SyncInfo(
                on_wait=waits[:1],
                on_update=list(si.on_update) if si.on_update else [],
            )
            for w in waits[1:]:
                extra = nc.sync.nop()
                extra.ins.sync_info = mybir.SyncInfo(on_wait=[w], on_update=[])
        nc.sync.drain()

        nc.all_engine_barrier()
        assert self.sems is not None
        popped = nc._tile_sem_poison_stack.pop()
        assert popped is self._sem_poison
        nc.clear_and_free_semaphores(list(self.sems.allocated().values()))
        nc.all_engine_barrier()

    tile_mod.TileContext._drain_and_barrier = _drain_and_barrier
    tile_mod.TileContext._ant_drain_patched = True


def _split_excess_waits(nc, max_waits=1):
    """This walrus build rejects instructions carrying more than `max_waits`
    sem waits ("Too many sync wait commands").  Move overflow waits onto
    same-engine NoOp instructions inserted immediately before the offender —
    per-engine program order makes this semantically identical."""
    import concourse.mybir as mybir

    ctr = [0]
    for f in nc.m.functions:
        for blk in f.blocks:
            il = blk.instructions
            out = []
            changed = False
            for ins in il:
                si = ins.sync_info
                waits = list(si.on_wait) if si is not None and si.on_wait else []
                if len(waits) > max_waits:
                    changed = True
                    extra = waits[:-max_waits]
                    for i0 in range(0, len(extra), max_waits):
                        nop = mybir.InstNoOp(
                            name=f"I-waitsplit-{ctr[0]}", ins=[], outs=[])
                        ctr[0] += 1
                        nop.engine = ins.engine
                        nop.sync_info = mybir.SyncInfo(
                            on_wait=extra[i0:i0 + max_waits], on_update=[])
                        out.append(nop)
                    ins.sync_info = mybir.SyncInfo(
                        on_wait=waits[-max_waits:],
                        on_update=list(si.on_update) if si.on_update else [],
                    )
                out.append(ins)
            if changed:
                blk.instructions = out


# ---------------------------------------------------------------------------
def _make_schedule(SB, NTn, sb_mt, tpb):
    """Interleaved step list: ('x', mt) / ('w', nr) /
    ('cell', sb, n, mts, nh) where mts is the tuple of mt-in-sb indices and
    nh is None (full nt) or 0/1 (half-width warmup micro-cell).

    Warmup: cell (0,0) split into four 2-mt x half-nt micro-cells so the
    first matmul needs only 4 quant chains (w0,w1,x0,x1) instead of 8.
    Stage A: wavefront over super-blocks 0-3 x all n-blocks (quadratic cell
    growth keeps the PE fed while the DVE builds quant supply).
    Stage B: rows sb=4..7 n-major (W fully quantized by then).  x-chains
    for sb reuse the xdqT window slots of sb-4, so they may only be emitted
    once all cells of sb-4 are scheduled.  One-cell supply lookahead.
    """
    A_SB = min(4, SB)
    ALL = tuple(range(sb_mt))
    LO, HI = ALL[:sb_mt // 2], ALL[sb_mt // 2:]
    cells = [(0, 0, LO, 0), (0, 0, HI, 0), (0, 0, LO, 1), (0, 0, HI, 1)]
    for w in range(max(A_SB, NTn)):
        wave = [(sb, n, ALL, None) for n in range(min(w + 1, NTn))
                for sb in range(min(w + 1, A_SB))
                if max(sb, n) == w and (sb, n) != (0, 0)]
        wave.sort(key=lambda c: (c[1], c[0]))
        cells.extend(wave)
    for sb in range(A_SB, SB):
        cells.extend((sb, n, ALL, None) for n in range(NTn))

    have_x, have_w = set(), set()
    done_per_sb = [0] * SB

    def supplies(cell):
        sb, n, mts, nh = cell
        half = tpb // 2
        nrs = (range(n * tpb, (n + 1) * tpb) if nh is None
               else range(n * tpb + nh * half, n * tpb + (nh + 1) * half))
        steps = []
        for nr in nrs:
            if nr not in have_w:
                have_w.add(nr)
                steps.append(('w', nr))
        for mt in [sb * sb_mt + mt for mt in mts]:
            if mt not in have_x:
                assert sb < 4 or done_per_sb[sb - 4] == 4 * NTn, (cell,)
                have_x.add(mt)
                steps.append(('x', mt))
        return steps

    sched = list(supplies(cells[0]))
    for k, cell in enumerate(cells):
        if k + 1 < len(cells):
            sched.extend(supplies(cells[k + 1]))
        sched.append(('cell',) + cell)
        done_per_sb[cell[0]] += 4 if cell[3] is None else 1
    return sched


def build_nc(m_core=M_CORE, k=K, n_core=N_CORE, num_cores=NUM_CORES,
             sb_mt=4, nt=512, kcg=6, debug=False, postprocess=True):
    """Build the per-core Bass program (SPMD: same program on every core)."""
    import concourse.bass as bass
    import concourse.mybir as mybir
    from concourse import tile
    from contextlib import ExitStack
    from collections import deque

    fp4_pre, fp4_fin = _register_custom_ops()
    _patch_tile_drain()

    KC = k // 128            # k-chunks (24)
    KB = k // 16             # scale blocks per row (192)
    NR = n_core // 128       # weight row-tiles (24)
    MT = m_core // 128       # x row-tiles (32)
    SB = MT // sb_mt         # super-blocks (8)
    NTn = n_core // nt       # n-blocks (6)
    TPB = nt // 128          # W row-tiles per n-block (4)
    NKG = KC // kcg          # wt DMA groups per cell (6)
    WIN = 4 * sb_mt          # xdqT window slots (16)
    assert m_core % (128 * sb_mt) == 0 and n_core % nt == 0 and KC % kcg == 0

    nc = bass.Bass("TRN2", target_bir_lowering=False, debug=debug,
                   num_devices=num_cores)
    dt = mybir.dt
    Alu = mybir.AluOpType

    x_d = nc.dram_tensor("x", [m_core, k], dt.float32, kind="ExternalInput")
    w_d = nc.dram_tensor("w", [n_core, k], dt.float32, kind="ExternalInput")
    b_d = nc.dram_tensor("bias", [n_core], dt.bfloat16, kind="ExternalInput")
    id_d = nc.dram_tensor("ident", [128, 128], dt.bfloat16, kind="ExternalInput")
    out_d = nc.dram_tensor("out", [m_core, n_core], dt.bfloat16, kind="ExternalOutput")

    with tile.TileContext(nc) as tc, ExitStack() as ctx:
        dram = ctx.enter_context(tc.tile_pool(name="dram", bufs=1, space="DRAM"))
        xin = ctx.enter_context(tc.tile_pool(name="xin", bufs=3))
        blk = ctx.enter_context(tc.tile_pool(name="blk", bufs=1))
        xdqp = ctx.enter_context(tc.tile_pool(name="xdqp", bufs=3))
        xT = ctx.enter_context(tc.tile_pool(name="xT", bufs=1))
        wTr = ctx.enter_context(tc.tile_pool(name="wTr", bufs=2))
        wst = ctx.enter_context(tc.tile_pool(name="wst", bufs=3))
        outp = ctx.enter_context(tc.tile_pool(name="outp", bufs=3))
        cst = ctx.enter_context(tc.tile_pool(name="cst", bufs=1))
        ps_mm2 = ctx.enter_context(tc.tile_pool(name="ps_mm2", bufs=2, space="PSUM"))
        ps_mm1 = ctx.enter_context(tc.tile_pool(name="ps_mm1", bufs=1, space="PSUM"))
        ps_tr = ctx.enter_context(tc.tile_pool(name="ps_tr", bufs=2, space="PSUM"))

        # wdqT stored n-major: 4 consecutive kc tiles of one n-block are
        # contiguous -> one [128, kcg*nt] load per kc-group.
        wdqT = dram.tile([NTn, KC, 128, nt], dt.bfloat16)

        ident = cst.tile([128, 128], dt.bfloat16, tag="ident")
        nc.sync.dma_start(out=ident[:, :], in_=id_d[:, :])
        # +inf per-partition scalar for FP4_FIN's exponent mask (an inf
        # *immediate* is not JSON-serializable through walrus)
        inf_t = cst.tile([128, 1], dt.float32, tag="inf")
        nc.vector.memset(inf_t[:, :], float("inf"))
        # bias broadcast DMA (768KB) is deferred until after the first quant
        # chain's input DMA so it doesn't delay the critical first chain
        bias_t = cst.tile([128, n_core], dt.bfloat16, tag="bias")
        _bias_loaded = [False]

        def _load_bias():
            if not _bias_loaded[0]:
                _bias_loaded[0] = True
                nc.sync.dma_start(
                    out=bias_t[:, :],
                    in_=b_d[:].unsqueeze(0).broadcast_to([128, n_core]),
                )

        # ---- quant chains (single 128-row tile each) ----------------------
        # All blk/qw intermediates are produced and consumed by the DVE
        # alone; its in-order execution makes bufs=1 stall-free.
        _pend = []           # chains whose reduce is emitted, smalls pending

        def _emit_scales(chains):
            width = len(chains) * KB
            bm = chains[0]['bt'][:, 0:width]
            sraw = blk.tile([128, 2 * KB], dt.float32, tag="sraw", name="sraw")[:, 0:width]
            nc.vector.tensor_scalar(
                sraw, bm, RCP6, float(2.0**-9), Alu.mult, Alu.max)
            pe = blk.tile([128, 2 * KB], dt.float32, tag="pe", name="pe")[:, 0:width]
            nc.vector.tensor_scalar(
                pe.bitcast(dt.int32), sraw.bitcast(dt.int32),
                0x7F800000, None, Alu.bitwise_and)
            pe2 = blk.tile([128, 2 * KB], dt.float32, tag="pe2", name="pe2")[:, 0:width]
            nc.vector.tensor_scalar_max(pe2, pe, float(2.0**-6))
            # pinv = 1/pe2 exactly: pe2 is a mantissa-zero power of two, so
            # bits(1/pe2) = 0x7F000000 - bits(pe2).  For x >= 0,
            # x ^ 0x7FFFFFFF == 0x7FFFFFFF - x, then subtract 0x00FFFFFF.
            pinv = blk.tile([128, 2 * KB], dt.float32, tag="pinv", name="pinv")[:, 0:width]
            pxr = blk.tile([128, 2 * KB], dt.float32, tag="pxr", name="pxr")[:, 0:width]
            nc.vector.tensor_scalar(
                pxr.bitcast(dt.int32), pe2.bitcast(dt.int32),
                0x7FFFFFFF, None, Alu.bitwise_xor)
            nc.vector.tensor_scalar(
                pinv.bitcast(dt.int32), pxr.bitcast(dt.int32),
                0x00FFFFFF, None, Alu.subtract)
            u = blk.tile([128, 2 * KB], dt.float32, tag="u", name="u")[:, 0:width]
            nc.vector.tensor_tensor(u, sraw, pinv, Alu.mult)
            wq = blk.tile([128, 2 * KB], dt.float32, tag="wq", name="wq")[:, 0:width]
            Cm = float(1.5 * 2**20)
            nc.vector.tensor_scalar(wq, u, Cm, -Cm, Alu.add, Alu.add)
            s = blk.tile([128, 2 * KB], dt.float32, tag="s", name="s")[:, 0:width]
            nc.vector.tensor_tensor(s, wq, pe2, Alu.mult)
            sh = blk.tile([128, 2 * KB], dt.float32, tag="sh", name="sh")[:, 0:width]
            nc.vector.tensor_scalar_mul(sh, s, 0.5)
            rinv = blk.tile([128, 2 * KB], dt.float32, tag="rinv", name="rinv")[:, 0:width]
            nc.vector.reciprocal(rinv, s)
            for ci, ch in enumerate(chains):
                ch['sh'] = sh[:, ci * KB:(ci + 1) * KB]
                ch['rinv'] = rinv[:, ci * KB:(ci + 1) * KB]

        def _finish_quant(ch):
            # fp4_pre runs IN-PLACE on the input tile (1:1 elementwise; the
            # DVE pipeline reads each element before writing it back), which
            # frees the qpre pool and pays for a third xin buffer so input
            # DMA runs ~3 chains ahead instead of stalling on buffer WAR.
            x3 = ch['xt'][:, :].rearrange("p (b e) -> p b e", e=16)
            nc.vector._custom_dve(
                fp4_pre, out=x3, in0=x3,
                in1=ch['rinv'].unsqueeze(2).broadcast_to([128, KB, 16]),
                s0=4.0, s1=CH1,
            )
            xdq = xdqp.tile([128, k], dt.bfloat16, tag="xdq", name="xdq")
            xdq3 = xdq[:, :].rearrange("p (b e) -> p b e", e=16)
            nc.vector._custom_dve(
                fp4_fin, out=xdq3, in0=x3,
                in1=ch['sh'].unsqueeze(2).broadcast_to([128, KB, 16]),
                s0=inf_t[:, 0:1], s1=12.0,
            )
            return xdq

        def quant_tile(kind, idx, last):
            src_d = x_d if kind == 'x' else w_d
            xt = xin.tile([128, k], dt.float32, tag="xin", name="xt")
            nc.sync.dma_start(out=xt[:, :], in_=src_d[idx * 128:idx * 128 + 128, :])
            x3 = xt[:, :].rearrange("p (b e) -> p b e", e=16)
            if not _pend:
                bt = blk.tile([128, 2 * KB], dt.float32, tag="bm", name="bm")
            else:
                bt = _pend[0]['bt']
            ci = len(_pend)
            nc.vector.tensor_reduce(
                bt[:, ci * KB:(ci + 1) * KB], x3, axis=mybir.AxisListType.X,
                op=Alu.max, apply_absolute_value=True,
            )
            _pend.append({'xt': xt, 'bt': bt, 'kind': kind, 'idx': idx})
            if len(_pend) == 2 or last:
                chains = list(_pend)
                _pend.clear()
                _emit_scales(chains)
                return [(c, _finish_quant(c)) for c in chains]
            return []

        # ---- transposes ---------------------------------------------------
        # Pending PE-transpose closures keyed by supply, paced into the
        # matmul stream.  The newest two chains' transposes are held back
        # during pacing (their DVE outputs are likely still computing);
        # a mandatory pre-cell drain guarantees dependencies.
        pending_T = deque()

        def emit_T(count):
            for _ in range(min(count, len(pending_T))):
                pending_T.popleft()[1]()

        def emit_T_for(keys):
            while pending_T and any(kv in keys for kv, _ in pending_T):
                pending_T.popleft()[1]()

        xT_slots = {}
        _tc = [0]

        TQ = 4               # transposes per PSUM tile / ACT evac
        NTQ = KC // TQ       # closures per chain (6)

        def _tr_quad(src, dst, kq):
            """Transpose kc in [kq*TQ, (kq+1)*TQ) of src into one [128, TQ*128]
            PSUM tile (disjoint column slots, each its own start/stop group),
            then one ACT evac to the contiguous dst range."""
            _tc[0] += 1
            pst = ps_tr.tile([128, TQ * 128], dt.bfloat16, tag="tr",
                             name=f"pst{_tc[0]}")
            for t in range(TQ):
                kc = kq * TQ + t
                nc.tensor.transpose(
                    pst[:, t * 128:(t + 1) * 128],
                    src[:, kc * 128:(kc + 1) * 128], ident[:, :])
            nc.scalar.copy(
                dst[:, kq * TQ * 128:(kq + 1) * TQ * 128], pst[:, :])

        def emit_x_tr(mt, xdq):
            xTt = xT.tile([128, k], dt.bfloat16, tag=f"s{mt % WIN}",
                          name=f"xTt{mt}")
            xT_slots[mt] = xTt
            for kq in range(NTQ):
                def run(xdq=xdq, xTt=xTt, kq=kq):
                    _tr_quad(xdq, xTt, kq)
                pending_T.append((('x', mt), run))

        def emit_w_tr(nr, wdq):
            wrow = wTr.tile([128, k], dt.bfloat16, tag="wTr", name=f"wTr{nr}")
            for kq in range(NTQ):
                def run(wdq=wdq, wrow=wrow, kq=kq, nr=nr, fin=(kq == NTQ - 1)):
                    _tr_quad(wdq, wrow, kq)
                    if fin:
                        # row-tile nr = 128-col block nr%TPB of every kc tile
                        # in n-block nr//TPB: one strided DMA
                        nbi, c0 = nr // TPB, (nr % TPB) * 128
                        nc.sync.dma_start(
                            out=wdqT[nbi, :, :, c0:c0 + 128].rearrange(
                                "t p c -> p t c"),
                            in_=wrow[:, :].rearrange("p (t c) -> p t c", t=KC))
                pending_T.append((('w', nr), run))

        def emit_chain(kind, idx, last=False):
            # Drain pending transposes down to one chain's worth before a
            # new chain can recycle an xdqp/wTr buffer (bufs=3: a tile 3
            # allocations back must have its reader closures emitted).
            emit_T(len(pending_T) - NTQ)
            for ch, xdq in quant_tile(kind, idx, last):
                if ch['kind'] == 'x':
                    emit_x_tr(ch['idx'], xdq)
                else:
                    emit_w_tr(ch['idx'], xdq)

        def emit_cell(sb, n, mts, nh):
            # nh=None: full nt-wide cell; nh=0/1: half-width warmup micro-cell
            nw = nt if nh is None else nt // 2
            c0 = n * nt + (0 if nh is None else nh * nw)
            nr_lo = c0 // 128
            need = {('x', sb * sb_mt + mt) for mt in mts}
            need |= {('w', nr_lo + t) for t in range(nw // 128)}
            emit_T_for(need)
            pmm = {}
            for mt in mts:
                pool = ps_mm2 if mt < 2 else ps_mm1
                pmm[mt] = pool.tile([128, nw], dt.float32, tag=f"mm{mt}",
                                    name=f"pmm{sb}_{n}_{mt}")
                # bias prefill: all matmuls accumulate onto it (start=False)
                nc.scalar.copy(pmm[mt][:, :], bias_t[:, c0:c0 + nw])
            xts = {mt: xT_slots[sb * sb_mt + mt] for mt in mts}
            # pace pending transposes, holding back the newest two chains
            drainable = max(0, len(pending_T) - 2 * NTQ)
            t_rate = (drainable + NKG - 1) // NKG
            coff = c0 - n * nt
            for g in range(NKG):
                wt = wst.tile([128, kcg * nw], dt.bfloat16, tag="wstream",
                              name=f"wt{sb}_{n}_{g}")
                nc.sync.dma_start(
                    out=wt[:, :].rearrange("p (t c) -> p t c", t=kcg),
                    in_=wdqT[n, g * kcg:(g + 1) * kcg, :,
                             coff:coff + nw].rearrange("t p c -> p t c"))
                emit_T(t_rate)
                for t in range(kcg):
                    kc = g * kcg + t
                    for mt in mts:
                        nc.tensor.matmul(
                            pmm[mt][:, :],
                            xts[mt][:, kc * 128:(kc + 1) * 128],
                            wt[:, t * nw:(t + 1) * nw],
                            start=False, stop=(kc == KC - 1),
                            skip_group_check=True,
                        )
            for mt in mts:
                r0 = (sb * sb_mt + mt) * 128
                ob = outp.tile([128, nw], dt.bfloat16, tag="ob", name="ob")
                nc.scalar.copy(ob[:, :], pmm[mt][:, :])
                nc.sync.dma_start(
                    out=out_d[r0:r0 + 128, c0:c0 + nw],
                    in_=ob[:, :])

        sched = _make_schedule(SB, NTn, sb_mt, TPB)
        n_chains = sum(1 for s in sched if s[0] in ('x', 'w'))
        assert n_chains == MT + NR
        seen = 0
        for step in sched:
            if step[0] == 'cell':
                emit_cell(*step[1:])
            else:
                seen += 1
                emit_chain(step[0], step[1], last=(seen == n_chains))
                _load_bias()
        emit_T(len(pending_T))
        assert not pending_T and not _pend

    if postprocess:
        _split_excess_waits(nc)
        # Raw Bass skips the ISA-byte encoding pass (Bacc.compile runs it);
        # without it custom-DVE/extended insts ship empty .instr -> walrus
        # "ISA wrong length".
        mybir.codegen_inst_isa_subclasses(nc)
    return nc


# ---------------------------------------------------------------------------
def _get_built():
    global _BUILT
    if _BUILT is None:
        _BUILT = build_nc()
    return _BUILT


def kernel(x, weight, bias):
    """Full-input entry point: x [2,4096,3072] f32, weight [12288,3072] f32,
    bias [12288] bf16 -> out [2,4096,12288] bf16."""
    from concourse.bass_utils import run_bass_kernel_spmd

    nc = _get_built()
    x2 = np.ascontiguousarray(np.asarray(x, dtype=f32).reshape(M, K))
    w = np.ascontiguousarray(np.asarray(weight, dtype=f32))
    b = np.asarray(bias)
    if b.dtype != bf16:
        if b.dtype.itemsize == 2 and b.dtype.kind in "Vu":
            b = b.view(bf16)
        else:
            b = b.astype(bf16)
    ident = np.eye(128, dtype=bf16)

    in_maps = []
    for c in range(NUM_CORES):
        mi, nj = divmod(c, GRID_N)
        in_maps.append({
            "x": x2[mi * M_CORE:(mi + 1) * M_CORE],
            "w": w[nj * N_CORE:(nj + 1) * N_CORE],
            "bias": b[nj * N_CORE:(nj + 1) * N_CORE],
            "ident": ident,
        })

    res = run_bass_kernel_spmd(nc, in_maps, list(range(NUM_CORES)))
    out = np.empty((M, N), dtype=bf16)
    for c in range(NUM_CORES):
        mi, nj = divmod(c, GRID_N)
        out[mi * M_CORE:(mi + 1) * M_CORE, nj * N_CORE:(nj + 1) * N_CORE] = (
            np.asarray(res.results[c]["out"]).astype(bf16, copy=False)
        )
    return out.reshape(B, T, N)


# revision 41
# speedup vs baseline: 1.0182x; 1.0182x over previous
"""NVFP4 block-quantized linear layer (x @ w.T + bias) on 8 Trainium2 cores.

Reference semantics (reference.py): both activations and weights are
block-quantized along K (blocks of 16) to fp4-e2m1 with e4m3 scales
(scale = absmax/6, round-to-nearest), dequantized, then matmul with fp32
accumulation, cast to bf16, plus bf16 bias.

Device strategy (per core, 2-way M x 4-way N grid), v2:
  - single-row-tile quant chains on VectorE (finer supply granularity),
    with the small per-block scale ops batched per two chains; exponent
    reciprocal via exact XOR/SUB bit trick (replaces a 3us RECIPROCAL).
  - build-time wavefront schedule interleaves quant chains and matmul
    cells so the PE does not starve on quant supply (the old sb-major
    order stalled ~450us waiting on W quant).
  - bias pre-filled into PSUM by ScalarE; matmuls accumulate onto it
    (start=False), dropping the DVE bias pass.
  - transposes grouped 4-per-PSUM-tile with one ScalarE evac each (4x
    fewer evacs; removes the PE-behind-ACT micro-stalls).
  - wdqT DRAM roundtrip in [NTn][KC][128,nt] layout: wt loads batched
    6-kc per DMA issue, W-transpose writebacks one strided DMA per row
    tile (3x fewer SP DMA issues).
  - xdqT resident in a 16-slot SBUF window (4 super-blocks); stage B
    super-blocks reuse slots via Tile WAR tracking.

Measured on 8 trn2 cores: ~1.42-1.47 ms HW exec (baseline 2.18 ms),
rel err ~3e-3 (tolerance 2e-2; the delta vs the old 1.6e-4 is the
single-rounded bias add from the PSUM prefill).
"""

import numpy as np
import ml_dtypes

f32 = np.float32
bf16 = ml_dtypes.bfloat16

# ---------------------------------------------------------------------------
# problem geometry (hardcoded; harness calls kernel() with these full shapes)
B, T, K = 2, 4096, 3072
N = 12288
M = B * T                      # 8192
GRID_M, GRID_N = 2, 4          # 8 cores
M_CORE = M // GRID_M           # 4096
N_CORE = N // GRID_N           # 3072
NUM_CORES = GRID_M * GRID_N

CH1 = float(1.5 * 2**22)
RCP6 = float(f32(1.0) / f32(6.0))

_BUILT = None


# ---------------------------------------------------------------------------
def _register_custom_ops():
    """Register the two fp4-rounding custom DVE ops (idempotent)."""
    import concourse.dve_ops as dve_ops
    from concourse.dve_ops import DveOp, OPS, _SUB_OPCODE_FOR_NAME, _CUSTOM_DVE_ROW_BASE
    from concourse.dve_spec import (
        Spec, Src0, Src1, C0, C1, Zero, One, AluOp, Bin,
        maxx, minn, select, lower, _has_src1,
    )
    from concourse.dve_uop import DveOpSpec

    def _norm2(in0, in1):
        in0 = np.asarray(in0)
        in1 = np.asarray(in1)
        if in1.size != in0.size:
            in1 = np.broadcast_to(in1, in0.shape)
        return in0, np.ascontiguousarray(in1).reshape(in0.shape)

    def _ref_fp4_pre(in0, in1, s0, s1, imm2=None):
        in0, in1 = _norm2(in0, in1)
        m = (in0.astype(f32) * in1.astype(f32)).astype(f32)
        s2 = (m * m).astype(f32)
        ch = np.where(
            s2 < f32(4.0), f32(CH1),
            ((f32(1.0) + (s2 >= f32(16.0)).astype(f32)) * f32(1.5 * 2**23)).astype(f32),
        ).astype(f32)
        return (m + ch).astype(f32)

    def _ref_fp4_fin(in0, in1, s0, s1, imm2=None):
        in0, in1 = _norm2(in0, in1)
        qpre = np.ascontiguousarray(in0.astype(f32))
        pe = (qpre.view(np.uint32) & np.uint32(0x7F800000)).view(f32)
        d1 = (qpre - pe).astype(f32)
        q2 = ((d1 + d1).astype(f32) - pe).astype(f32)
        qc = np.maximum(np.minimum(q2, f32(12.0)), f32(-12.0))
        return (qc * in1.astype(f32)).astype(f32)

    def build_pre():
        SIXTEEN = C0 * C0
        Ch2x = C1 + C1
        m = Src0 * Src1
        s2 = m * m
        c2 = s2 >= SIXTEEN
        inner = (c2 + One) * Ch2x
        c1 = s2 < C0
        outer = select(c1, C1, inner)
        return Spec(body=m + outer, reference=_ref_fp4_pre)

    def build_fin():
        pe = Bin(AluOp.BITWISE_AND, Src0, C0)
        d1 = Src0 - pe
        q2 = (d1 + d1) - pe
        qc = maxx(minn(q2, C1), Zero - C1)
        return Spec(body=qc * Src1, reference=_ref_fp4_fin)

    def register(name, spec):
        if name in _SUB_OPCODE_FOR_NAME:
            for op in OPS:
                if op.name == name:
                    return op
            raise RuntimeError(name)
        row = _CUSTOM_DVE_ROW_BASE + len(OPS)
        assert row < 0x20
        shas = {}
        for ver in ("v3", "v4"):
            try:
                uops = lower(spec, ver=ver)
            except Exception:
                continue
            shas[ver] = DveOpSpec(
                name=name, opcode=row, uops=uops, rd1_en=_has_src1(spec)
            ).sha(ver)
        op = DveOp(name, spec, subdim=False, uops_sha=shas)
        OPS.append(op)
        _SUB_OPCODE_FOR_NAME[name] = row
        dve_ops.CUSTOM_DVE_SPECS[name] = spec
        return op

    return register("FP4_PRE_ANT", build_pre()), register("FP4_FIN_ANT", build_fin())


# ---------------------------------------------------------------------------
def _patch_tile_drain():
    """The TileContext tail drain attaches one sem-wait per live logical
    processor to a single SP Drain instruction; this walrus build caps sync
    waits per instruction at 2 ("Too many sync wait commands").  Split the
    overflow waits onto preceding single-wait SP nops (sound: all waits still
    complete before the post-drain all-engine barrier / sem reset)."""
    from concourse import tile as tile_mod
    import concourse.mybir as mybir
    from concourse.vector_clock import ScopedClock

    if getattr(tile_mod.TileContext, "_ant_drain_patched", False):
        return

    def _drain_and_barrier(self, tick_clock, wait_clock):
        nc = self.nc
        probe = nc.sync.nop()
        wait_clock.add_sem_waits(
            probe.ins, ScopedClock({None: tick_clock.global_clock})
        )
        si = probe.ins.sync_info
        waits = list(si.on_wait) if si is not None and si.on_wait else []
        if len(waits) > 1:
            probe.ins.sync_info = mybir.SyncInfo(
                on_wait=waits[:1],
                on_update=list(si.on_update) if si.on_update else [],
            )
            for w in waits[1:]:
                extra = nc.sync.nop()
                extra.ins.sync_info = mybir.SyncInfo(on_wait=[w], on_update=[])
        nc.sync.drain()

        nc.all_engine_barrier()
        assert self.sems is not None
        popped = nc._tile_sem_poison_stack.pop()
        assert popped is self._sem_poison
        nc.clear_and_free_semaphores(list(self.sems.allocated().values()))
        nc.all_engine_barrier()

    tile_mod.TileContext._drain_and_barrier = _drain_and_barrier
    tile_mod.TileContext._ant_drain_patched = True


def _split_excess_waits(nc, max_waits=1):
    """This walrus build rejects instructions carrying more than `max_waits`
    sem waits ("Too many sync wait commands").  Move overflow waits onto
    same-engine NoOp instructions inserted immediately before the offender —
    per-engine program order makes this semantically identical."""
    import concourse.mybir as mybir

    ctr = [0]
    for f in nc.m.functions:
        for blk in f.blocks:
            il = blk.instructions
            out = []
            changed = False
            for ins in il:
                si = ins.sync_info
                waits = list(si.on_wait) if si is not None and si.on_wait else []
                if len(waits) > max_waits:
                    changed = True
                    extra = waits[:-max_waits]
                    for i0 in range(0, len(extra), max_waits):
                        nop = mybir.InstNoOp(
                            name=f"I-waitsplit-{ctr[0]}", ins=[], outs=[])
                        ctr[0] += 1
                        nop.engine = ins.engine
                        nop.sync_info = mybir.SyncInfo(
                            on_wait=extra[i0:i0 + max_waits], on_update=[])
                        out.append(nop)
                    ins.sync_info = mybir.SyncInfo(
                        on_wait=waits[-max_waits:],
                        on_update=list(si.on_update) if si.on_update else [],
                    )
                out.append(ins)
            if changed:
                blk.instructions = out


# ---------------------------------------------------------------------------
def _make_schedule(SB, NTn, sb_mt, tpb):
    """Interleaved step list: ('x', mt) / ('w', nr) /
    ('cell', sb, n, mts, nh) where mts is the tuple of mt-in-sb indices and
    nh is None (full nt) or 0/1 (half-width warmup micro-cell).

    Warmup: cell (0,0) split into four 2-mt x half-nt micro-cells so the
    first matmul needs only 4 quant chains (w0,w1,x0,x1) instead of 8.
    Stage A: wavefront over super-blocks 0-3 x all n-blocks (quadratic cell
    growth keeps the PE fed while the DVE builds quant supply).
    Stage B: rows sb=4..7 n-major (W fully quantized by then).  x-chains
    for sb reuse the xdqT window slots of sb-4, so they may only be emitted
    once all cells of sb-4 are scheduled.  One-cell supply lookahead.
    """
    A_SB = min(4, SB)
    ALL = tuple(range(sb_mt))
    LO, HI = ALL[:sb_mt // 2], ALL[sb_mt // 2:]
    cells = [(0, 0, LO, 0), (0, 0, HI, 0), (0, 0, LO, 1), (0, 0, HI, 1)]
    for w in range(max(A_SB, NTn)):
        wave = [(sb, n, ALL, None) for n in range(min(w + 1, NTn))
                for sb in range(min(w + 1, A_SB))
                if max(sb, n) == w and (sb, n) != (0, 0)]
        wave.sort(key=lambda c: (c[1], c[0]))
        cells.extend(wave)
    for sb in range(A_SB, SB):
        cells.extend((sb, n, ALL, None) for n in range(NTn))

    have_x, have_w = set(), set()
    done_per_sb = [0] * SB

    def supplies(cell):
        sb, n, mts, nh = cell
        half = tpb // 2
        nrs = (range(n * tpb, (n + 1) * tpb) if nh is None
               else range(n * tpb + nh * half, n * tpb + (nh + 1) * half))
        steps = []
        for nr in nrs:
            if nr not in have_w:
                have_w.add(nr)
                steps.append(('w', nr))
        for mt in [sb * sb_mt + mt for mt in mts]:
            if mt not in have_x:
                assert sb < 4 or done_per_sb[sb - 4] == 4 * NTn, (cell,)
                have_x.add(mt)
                steps.append(('x', mt))
        return steps

    sched = list(supplies(cells[0]))
    sched.extend(supplies(cells[1]))
    for k, cell in enumerate(cells):
        # two-cell supply lookahead keeps the DVE queue non-empty while
        # cells run (it is the bottleneck engine in the warmup phase)
        if k + 2 < len(cells):
            sched.extend(supplies(cells[k + 2]))
        sched.append(('cell',) + cell)
        done_per_sb[cell[0]] += 4 if cell[3] is None else 1
    return sched


def build_nc(m_core=M_CORE, k=K, n_core=N_CORE, num_cores=NUM_CORES,
             sb_mt=4, nt=512, kcg=6, debug=False, postprocess=True):
    """Build the per-core Bass program (SPMD: same program on every core)."""
    import concourse.bass as bass
    import concourse.mybir as mybir
    from concourse import tile
    from contextlib import ExitStack
    from collections import deque

    fp4_pre, fp4_fin = _register_custom_ops()
    _patch_tile_drain()

    KC = k // 128            # k-chunks (24)
    KB = k // 16             # scale blocks per row (192)
    NR = n_core // 128       # weight row-tiles (24)
    MT = m_core // 128       # x row-tiles (32)
    SB = MT // sb_mt         # super-blocks (8)
    NTn = n_core // nt       # n-blocks (6)
    TPB = nt // 128          # W row-tiles per n-block (4)
    NKG = KC // kcg          # wt DMA groups per cell (6)
    WIN = 4 * sb_mt          # xdqT window slots (16)
    assert m_core % (128 * sb_mt) == 0 and n_core % nt == 0 and KC % kcg == 0

    nc = bass.Bass("TRN2", target_bir_lowering=False, debug=debug,
                   num_devices=num_cores)
    dt = mybir.dt
    Alu = mybir.AluOpType

    x_d = nc.dram_tensor("x", [m_core, k], dt.float32, kind="ExternalInput")
    w_d = nc.dram_tensor("w", [n_core, k], dt.float32, kind="ExternalInput")
    b_d = nc.dram_tensor("bias", [n_core], dt.bfloat16, kind="ExternalInput")
    id_d = nc.dram_tensor("ident", [128, 128], dt.bfloat16, kind="ExternalInput")
    out_d = nc.dram_tensor("out", [m_core, n_core], dt.bfloat16, kind="ExternalOutput")

    with tile.TileContext(nc) as tc, ExitStack() as ctx:
        dram = ctx.enter_context(tc.tile_pool(name="dram", bufs=1, space="DRAM"))
        xin = ctx.enter_context(tc.tile_pool(name="xin", bufs=3))
        blk = ctx.enter_context(tc.tile_pool(name="blk", bufs=1))
        xdqp = ctx.enter_context(tc.tile_pool(name="xdqp", bufs=3))
        xT = ctx.enter_context(tc.tile_pool(name="xT", bufs=1))
        wTr = ctx.enter_context(tc.tile_pool(name="wTr", bufs=2))
        wst = ctx.enter_context(tc.tile_pool(name="wst", bufs=3))
        outp = ctx.enter_context(tc.tile_pool(name="outp", bufs=3))
        cst = ctx.enter_context(tc.tile_pool(name="cst", bufs=1))
        ps_mm2 = ctx.enter_context(tc.tile_pool(name="ps_mm2", bufs=2, space="PSUM"))
        ps_mm1 = ctx.enter_context(tc.tile_pool(name="ps_mm1", bufs=1, space="PSUM"))
        ps_tr = ctx.enter_context(tc.tile_pool(name="ps_tr", bufs=2, space="PSUM"))

        # wdqT stored n-major: 4 consecutive kc tiles of one n-block are
        # contiguous -> one [128, kcg*nt] load per kc-group.
        wdqT = dram.tile([NTn, KC, 128, nt], dt.bfloat16)

        ident = cst.tile([128, 128], dt.bfloat16, tag="ident")
        nc.sync.dma_start(out=ident[:, :], in_=id_d[:, :])
        # +inf per-partition scalar for FP4_FIN's exponent mask (an inf
        # *immediate* is not JSON-serializable through walrus)
        inf_t = cst.tile([128, 1], dt.float32, tag="inf")
        nc.vector.memset(inf_t[:, :], float("inf"))
        # bias broadcast DMA (768KB) is deferred until after the first quant
        # chain's input DMA so it doesn't delay the critical first chain
        bias_t = cst.tile([128, n_core], dt.bfloat16, tag="bias")
        _bias_loaded = [False]

        def _load_bias():
            if not _bias_loaded[0]:
                _bias_loaded[0] = True
                nc.sync.dma_start(
                    out=bias_t[:, :],
                    in_=b_d[:].unsqueeze(0).broadcast_to([128, n_core]),
                )

        # ---- quant chains (single 128-row tile each) ----------------------
        # All blk/qw intermediates are produced and consumed by the DVE
        # alone; its in-order execution makes bufs=1 stall-free.
        _pend = []           # chains whose reduce is emitted, smalls pending

        def _emit_scales(chains):
            width = len(chains) * KB
            bm = chains[0]['bt'][:, 0:width]
            sraw = blk.tile([128, 2 * KB], dt.float32, tag="sraw", name="sraw")[:, 0:width]
            nc.vector.tensor_scalar(
                sraw, bm, RCP6, float(2.0**-9), Alu.mult, Alu.max)
            pe = blk.tile([128, 2 * KB], dt.float32, tag="pe", name="pe")[:, 0:width]
            nc.vector.tensor_scalar(
                pe.bitcast(dt.int32), sraw.bitcast(dt.int32),
                0x7F800000, None, Alu.bitwise_and)
            pe2 = blk.tile([128, 2 * KB], dt.float32, tag="pe2", name="pe2")[:, 0:width]
            nc.vector.tensor_scalar_max(pe2, pe, float(2.0**-6))
            # pinv = 1/pe2 exactly: pe2 is a mantissa-zero power of two, so
            # bits(1/pe2) = 0x7F000000 - bits(pe2).  For x >= 0,
            # x ^ 0x7FFFFFFF == 0x7FFFFFFF - x, then subtract 0x00FFFFFF.
            pinv = blk.tile([128, 2 * KB], dt.float32, tag="pinv", name="pinv")[:, 0:width]
            pxr = blk.tile([128, 2 * KB], dt.float32, tag="pxr", name="pxr")[:, 0:width]
            nc.vector.tensor_scalar(
                pxr.bitcast(dt.int32), pe2.bitcast(dt.int32),
                0x7FFFFFFF, None, Alu.bitwise_xor)
            nc.vector.tensor_scalar(
                pinv.bitcast(dt.int32), pxr.bitcast(dt.int32),
                0x00FFFFFF, None, Alu.subtract)
            u = blk.tile([128, 2 * KB], dt.float32, tag="u", name="u")[:, 0:width]
            nc.vector.tensor_tensor(u, sraw, pinv, Alu.mult)
            wq = blk.tile([128, 2 * KB], dt.float32, tag="wq", name="wq")[:, 0:width]
            Cm = float(1.5 * 2**20)
            nc.vector.tensor_scalar(wq, u, Cm, -Cm, Alu.add, Alu.add)
            s = blk.tile([128, 2 * KB], dt.float32, tag="s", name="s")[:, 0:width]
            nc.vector.tensor_tensor(s, wq, pe2, Alu.mult)
            sh = blk.tile([128, 2 * KB], dt.float32, tag="sh", name="sh")[:, 0:width]
            nc.vector.tensor_scalar_mul(sh, s, 0.5)
            rinv = blk.tile([128, 2 * KB], dt.float32, tag="rinv", name="rinv")[:, 0:width]
            nc.vector.reciprocal(rinv, s)
            for ci, ch in enumerate(chains):
                ch['sh'] = sh[:, ci * KB:(ci + 1) * KB]
                ch['rinv'] = rinv[:, ci * KB:(ci + 1) * KB]

        def _finish_quant(ch):
            # fp4_pre runs IN-PLACE on the input tile (1:1 elementwise; the
            # DVE pipeline reads each element before writing it back), which
            # frees the qpre pool and pays for a third xin buffer so input
            # DMA runs ~3 chains ahead instead of stalling on buffer WAR.
            x3 = ch['xt'][:, :].rearrange("p (b e) -> p b e", e=16)
            nc.vector._custom_dve(
                fp4_pre, out=x3, in0=x3,
                in1=ch['rinv'].unsqueeze(2).broadcast_to([128, KB, 16]),
                s0=4.0, s1=CH1,
            )
            xdq = xdqp.tile([128, k], dt.bfloat16, tag="xdq", name="xdq")
            xdq3 = xdq[:, :].rearrange("p (b e) -> p b e", e=16)
            nc.vector._custom_dve(
                fp4_fin, out=xdq3, in0=x3,
                in1=ch['sh'].unsqueeze(2).broadcast_to([128, KB, 16]),
                s0=inf_t[:, 0:1], s1=12.0,
            )
            return xdq

        def quant_tile(kind, idx, last):
            src_d = x_d if kind == 'x' else w_d
            xt = xin.tile([128, k], dt.float32, tag="xin", name="xt")
            nc.sync.dma_start(out=xt[:, :], in_=src_d[idx * 128:idx * 128 + 128, :])
            x3 = xt[:, :].rearrange("p (b e) -> p b e", e=16)
            if not _pend:
                bt = blk.tile([128, 2 * KB], dt.float32, tag="bm", name="bm")
            else:
                bt = _pend[0]['bt']
            ci = len(_pend)
            nc.vector.tensor_reduce(
                bt[:, ci * KB:(ci + 1) * KB], x3, axis=mybir.AxisListType.X,
                op=Alu.max, apply_absolute_value=True,
            )
            _pend.append({'xt': xt, 'bt': bt, 'kind': kind, 'idx': idx})
            if len(_pend) == 2 or last:
                chains = list(_pend)
                _pend.clear()
                _emit_scales(chains)
                return [(c, _finish_quant(c)) for c in chains]
            return []

        # ---- transposes ---------------------------------------------------
        # Pending PE-transpose closures keyed by supply, paced into the
        # matmul stream.  The newest two chains' transposes are held back
        # during pacing (their DVE outputs are likely still computing);
        # a mandatory pre-cell drain guarantees dependencies.
        pending_T = deque()

        def emit_T(count):
            for _ in range(min(count, len(pending_T))):
                pending_T.popleft()[1]()

        def emit_T_for(keys):
            while pending_T and any(kv in keys for kv, _ in pending_T):
                pending_T.popleft()[1]()

        xT_slots = {}
        _tc = [0]

        TQ = 4               # transposes per PSUM tile / ACT evac
        NTQ = KC // TQ       # closures per chain (6)

        def _tr_quad(src, dst, kq):
            """Transpose kc in [kq*TQ, (kq+1)*TQ) of src into one [128, TQ*128]
            PSUM tile (disjoint column slots, each its own start/stop group),
            then one ACT evac to the contiguous dst range."""
            _tc[0] += 1
            pst = ps_tr.tile([128, TQ * 128], dt.bfloat16, tag="tr",
                             name=f"pst{_tc[0]}")
            for t in range(TQ):
                kc = kq * TQ + t
                nc.tensor.transpose(
                    pst[:, t * 128:(t + 1) * 128],
                    src[:, kc * 128:(kc + 1) * 128], ident[:, :])
            nc.scalar.copy(
                dst[:, kq * TQ * 128:(kq + 1) * TQ * 128], pst[:, :])

        def emit_x_tr(mt, xdq):
            xTt = xT.tile([128, k], dt.bfloat16, tag=f"s{mt % WIN}",
                          name=f"xTt{mt}")
            xT_slots[mt] = xTt
            for kq in range(NTQ):
                def run(xdq=xdq, xTt=xTt, kq=kq):
                    _tr_quad(xdq, xTt, kq)
                pending_T.append((('x', mt), run))

        def emit_w_tr(nr, wdq):
            wrow = wTr.tile([128, k], dt.bfloat16, tag="wTr", name=f"wTr{nr}")
            for kq in range(NTQ):
                def run(wdq=wdq, wrow=wrow, kq=kq, nr=nr, fin=(kq == NTQ - 1)):
                    _tr_quad(wdq, wrow, kq)
                    if fin:
                        # row-tile nr = 128-col block nr%TPB of every kc tile
                        # in n-block nr//TPB: one strided DMA
                        nbi, c0 = nr // TPB, (nr % TPB) * 128
                        nc.sync.dma_start(
                            out=wdqT[nbi, :, :, c0:c0 + 128].rearrange(
                                "t p c -> p t c"),
                            in_=wrow[:, :].rearrange("p (t c) -> p t c", t=KC))
                pending_T.append((('w', nr), run))

        def emit_chain(kind, idx, last=False):
            # Drain pending transposes down to one chain's worth before a
            # new chain can recycle an xdqp/wTr buffer (bufs=3: a tile 3
            # allocations back must have its reader closures emitted).
            emit_T(len(pending_T) - NTQ)
            for ch, xdq in quant_tile(kind, idx, last):
                if ch['kind'] == 'x':
                    emit_x_tr(ch['idx'], xdq)
                else:
                    emit_w_tr(ch['idx'], xdq)

        def emit_cell(sb, n, mts, nh):
            # nh=None: full nt-wide cell; nh=0/1: half-width warmup micro-cell
            nw = nt if nh is None else nt // 2
            c0 = n * nt + (0 if nh is None else nh * nw)
            nr_lo = c0 // 128
            need = {('x', sb * sb_mt + mt) for mt in mts}
            need |= {('w', nr_lo + t) for t in range(nw // 128)}
            emit_T_for(need)
            pmm = {}
            for mt in mts:
                pool = ps_mm2 if mt < 2 else ps_mm1
                pmm[mt] = pool.tile([128, nw], dt.float32, tag=f"mm{mt}",
                                    name=f"pmm{sb}_{n}_{mt}")
                # bias prefill: all matmuls accumulate onto it (start=False)
                nc.scalar.copy(pmm[mt][:, :], bias_t[:, c0:c0 + nw])
            xts = {mt: xT_slots[sb * sb_mt + mt] for mt in mts}
            # pace pending transposes, holding back the newest two chains
            drainable = max(0, len(pending_T) - 2 * NTQ)
            t_rate = (drainable + NKG - 1) // NKG
            coff = c0 - n * nt
            for g in range(NKG):
                wt = wst.tile([128, kcg * nw], dt.bfloat16, tag="wstream",
                              name=f"wt{sb}_{n}_{g}")
                nc.sync.dma_start(
                    out=wt[:, :].rearrange("p (t c) -> p t c", t=kcg),
                    in_=wdqT[n, g * kcg:(g + 1) * kcg, :,
                             coff:coff + nw].rearrange("t p c -> p t c"))
                emit_T(t_rate)
                for t in range(kcg):
                    kc = g * kcg + t
                    for mt in mts:
                        nc.tensor.matmul(
                            pmm[mt][:, :],
                            xts[mt][:, kc * 128:(kc + 1) * 128],
                            wt[:, t * nw:(t + 1) * nw],
                            start=False, stop=(kc == KC - 1),
                            skip_group_check=True,
                        )
            for mt in mts:
                r0 = (sb * sb_mt + mt) * 128
                ob = outp.tile([128, nw], dt.bfloat16, tag="ob", name="ob")
                nc.scalar.copy(ob[:, :], pmm[mt][:, :])
                nc.sync.dma_start(
                    out=out_d[r0:r0 + 128, c0:c0 + nw],
                    in_=ob[:, :])

        sched = _make_schedule(SB, NTn, sb_mt, TPB)
        n_chains = sum(1 for s in sched if s[0] in ('x', 'w'))
        assert n_chains == MT + NR
        seen = 0
        for step in sched:
            if step[0] == 'cell':
                emit_cell(*step[1:])
            else:
                seen += 1
                emit_chain(step[0], step[1], last=(seen == n_chains))
                _load_bias()
        emit_T(len(pending_T))
        assert not pending_T and not _pend

    if postprocess:
        _split_excess_waits(nc)
        # Raw Bass skips the ISA-byte encoding pass (Bacc.compile runs it);
        # without it custom-DVE/extended insts ship empty .instr -> walrus
        # "ISA wrong length".
        mybir.codegen_inst_isa_subclasses(nc)
    return nc


# ---------------------------------------------------------------------------
def _get_built():
    global _BUILT
    if _BUILT is None:
        _BUILT = build_nc()
    return _BUILT


def kernel(x, weight, bias):
    """Full-input entry point: x [2,4096,3072] f32, weight [12288,3072] f32,
    bias [12288] bf16 -> out [2,4096,12288] bf16."""
    from concourse.bass_utils import run_bass_kernel_spmd

    nc = _get_built()
    x2 = np.ascontiguousarray(np.asarray(x, dtype=f32).reshape(M, K))
    w = np.ascontiguousarray(np.asarray(weight, dtype=f32))
    b = np.asarray(bias)
    if b.dtype != bf16:
        if b.dtype.itemsize == 2 and b.dtype.kind in "Vu":
            b = b.view(bf16)
        else:
            b = b.astype(bf16)
    ident = np.eye(128, dtype=bf16)

    in_maps = []
    for c in range(NUM_CORES):
        mi, nj = divmod(c, GRID_N)
        in_maps.append({
            "x": x2[mi * M_CORE:(mi + 1) * M_CORE],
            "w": w[nj * N_CORE:(nj + 1) * N_CORE],
            "bias": b[nj * N_CORE:(nj + 1) * N_CORE],
            "ident": ident,
        })

    res = run_bass_kernel_spmd(nc, in_maps, list(range(NUM_CORES)))
    out = np.empty((M, N), dtype=bf16)
    for c in range(NUM_CORES):
        mi, nj = divmod(c, GRID_N)
        out[mi * M_CORE:(mi + 1) * M_CORE, nj * N_CORE:(nj + 1) * N_CORE] = (
            np.asarray(res.results[c]["out"]).astype(bf16, copy=False)
        )
    return out.reshape(B, T, N)
